# revision 1
# baseline (speedup 1.0000x reference)
"""ConditionAwareAdaIN Trainium2 kernel.

Reference computation (B=16, C=256, L=1024, U=64, Q=64):
    nx    = InstanceNorm1d(x)                       # per-(b,c) stats over L
    A     = einsum('bu,cuq->bcq', u_i, W.reshape(2C,U,Q))
    style = einsum('bcq,bql->bcl', A, e_qid)
    gamma, beta = split(style + V@t + bias, 2, axis=1)
    out   = (1 + gamma) * nx + beta

Sharding: 2-way over batch x 4-way over channels -> 8 cores, each owning
8 samples x 64 channels (its slice of gamma AND beta rows of W/V/bias).

Per-core device kernel:
  stage 1: A[b,(q,c2)] via a batched matmul over K=u (host-pretransposed W),
           PSUM evacuated by ACT/DVE, bounced through DRAM into per-pair
           A_T tiles (q on partitions).
  stage 2: per sample-pair, block-diagonal style matmuls (K=128) + a K=3
           accumulating matmul folding V*t, bias and the "+1" of (1+gamma).
  norm:    bn_stats/bn_aggr per pair tile (2 samples x 64 ch = 128 rows),
           rstd folded into the multiplier during PSUM evacuation (ACT),
           then out = (x - mean) * M'' + beta on DVE.

float32r is used for all matmul operands (full-rate PE streaming, ~tf32
rounding; end-to-end rel err ~1e-4).
"""

import json

import numpy as np

for _p in ("/opt/trn_rl_repo", "/root/.axon_site/_ro/trn_rl_repo"):
    import sys as _sys

    if _p not in _sys.path:
        _sys.path.append(_p)

import concourse.bass as bass
import concourse.mybir as mybir
from concourse.tile import TileContext
from concourse.bass_utils import run_bass_kernel_spmd


def _split_sync_waits(raw: bytes, keep: int = 1) -> bytes:
    """Walrus in this env accepts at most one sync wait per TPB instruction.

    Tile packs several waits into sync_info.on_wait; re-emit the excess as
    standalone single-wait EventSemaphore instructions (what wait_ge emits)
    immediately before the instruction, in the same engine stream.
    """
    bir = json.loads(raw)
    n = 0
    for fn in bir["functions"]:
        for blk in fn["blocks"]:
            out = []
            for ins in blk["instructions"]:
                si = ins.get("sync_info")
                ws = si.get("on_wait") if si else None
                if ws and len(ws) > keep:
                    for w in ws[: len(ws) - keep]:
                        n += 1
                        out.append(
                            {
                                "debug": ins.get("debug", 0),
                                "engine": ins["engine"],
                                "ins": [],
                                "outs": [],
                                "name": f"evw-{n}",
                                "opcode": "EventSemaphore",
                                "sync_info": {"on_update": [], "on_wait": [w]},
                            }
                        )
                    si["on_wait"] = ws[len(ws) - keep :]
                out.append(ins)
            blk["instructions"] = out
    return json.dumps(bir).encode()


class _Bass(bass.Bass):
    def to_json_bytes(self) -> bytes:
        return _split_sync_waits(super().to_json_bytes())


B, C, L = 16, 256, 1024
U, Q = 64, 64
EPS = 1e-5
N_CORES = 8
BG, CG = 2, 4          # batch groups x channel groups
BPC = B // BG          # samples per core = 8
CPC = C // CG          # channels per core = 64
NPAIR = BPC // 2       # sample pairs per core = 4

FP32 = mybir.dt.float32

_CACHE = {}


def _build_nc(detect_races: bool = True):
    nc = _Bass(detect_race_conditions=detect_races)

    # xe: per pair, rows = (2 samples x 64 ch/q), cols = [x (1024) | e (1024)]
    xe_in = nc.dram_tensor("xe_s", [NPAIR, 128, 2 * L], FP32, kind="ExternalInput")
    # wt2 columns: [ui (8) | W-permuted (4096)] -- one tensor, one DMA per half
    wt_in = nc.dram_tensor("wt2", [128, BPC + 4096], FP32, kind="ExternalInput")
    # sm: [r2 (4 pairs x 1024) | l2 (256)] on 3 partitions
    sm_in = nc.dram_tensor("sm2", [3, NPAIR * L + 256], FP32, kind="ExternalInput")
    out_d = nc.dram_tensor("out_s", [BPC, CPC, L], FP32, kind="ExternalOutput")
    # DRAM bounce for the A redistribute: partition-offset SBUF DMA sources
    # flat-decode the offset into the free dim, so go through DRAM instead.
    a_dram = nc.dram_tensor("a_scratch", [BPC, Q, 2, CPC], FP32)

    AF = mybir.ActivationFunctionType
    OP = mybir.AluOpType
    F32R = mybir.dt.float32r

    with TileContext(nc) as tc:
        with (
            tc.tile_pool(name="persist", bufs=1) as persist,
            tc.tile_pool(name="xe", bufs=4) as xe,
            tc.tile_pool(name="work", bufs=4) as work,
            tc.tile_pool(name="stat", bufs=8) as stat,
            tc.tile_pool(name="ps", bufs=4, space="PSUM") as ps,
        ):
            sm = persist.tile([3, NPAIR * L + 256], F32R, tag="sm")
            nc.sync.dma_start(out=sm, in_=sm_in[:, :].bitcast(F32R))
            r2 = sm[:, 0 : NPAIR * L].rearrange("k (s l) -> k s l", s=NPAIR)
            l2 = sm[:, NPAIR * L : NPAIR * L + 256]
            # wt in two halves so stage-1 (qh=0) can start after the first one;
            # each half carries its own ui rows in cols 0:BPC
            wt = persist.tile([128, BPC + 4096], F32R, tag="wt")
            nc.sync.dma_start(out=wt[0:64, :], in_=wt_in[0:64, :].bitcast(F32R))
            nc.sync.dma_start(out=wt[64:128, :], in_=wt_in[64:128, :].bitcast(F32R))
            ui = wt[:, 0:BPC]

            a_sb = persist.tile([BPC, Q * 128], FP32, tag="a_sb")
            # block-diagonal lhsT, all pairs in one tile; per pair s the
            # free col = gb*128 + sp*64 + c':
            #   lt_all[sp*64+q, s, gb*128+sp'*64+c'] = A[2s+sp, gb*64+c', q] iff sp'==sp
            lt_all = persist.tile([128, NPAIR, 256], F32R, tag="lt_all")
            nc.gpsimd.memset(lt_all[:, :, :].bitcast(FP32), 0.0)
            eps_t = persist.tile([128, 1], FP32, tag="eps")
            nc.vector.memset(eps_t, EPS)

            # PE warm-up: dummy matmuls on early-arriving tiles keep the PE
            # HAM ramping while wt streams in, so stage-1 runs at full clock.
            for wu in range(4):
                pw = ps.tile([BPC, 512], FP32, tag="ps", name=f"pw{wu}")
                nc.tensor.matmul(
                    pw, lhsT=sm[:, 0:BPC], rhs=sm[:, 0:512], start=True, stop=True
                )

            # ---- stage 1: A[b, (q, c2)] = sum_u u_i[b,u] * Wr[c2,u,q] ----
            # wt rows (qh,u); free (ql, c2).  8 matmul groups of (8, 1024).
            for g in range(8):
                qh, ns = divmod(g, 4)
                pa = ps.tile([BPC, 1024], FP32, tag="ps", name=f"pa{g}")
                ks = slice(qh * 64, qh * 64 + 64)
                for h in range(2):
                    nc.tensor.matmul(
                        pa[:, h * 512 : (h + 1) * 512],
                        lhsT=ui[ks, :],
                        rhs=wt[ks, BPC + ns * 1024 + h * 512 : BPC + ns * 1024 + (h + 1) * 512],
                        start=True,
                        stop=True,
                    )
                dst = a_sb[:, qh * 4096 + ns * 1024 : qh * 4096 + (ns + 1) * 1024]
                if g % 2 == 0:
                    nc.scalar.activation(out=dst, in_=pa, func=AF.Copy)
                else:
                    nc.vector.tensor_copy(out=dst, in_=pa)

            # ---- redistribute A into per-pair A_T tiles (via DRAM) ----
            nc.sync.dma_start(out=a_dram[:, :, :, :], in_=a_sb[:, :])
            # two fills: one per sample-in-pair position (even b / odd b);
            # source order (q, s, gb, c) matches the dest AP flattening
            lt4 = lt_all.rearrange("p s (gb c) -> p s gb c", gb=2)
            av = a_dram.rearrange("(s two) q gb c -> two q s gb c", two=2)
            for sp in range(2):
                rows = slice(sp * 64, sp * 64 + 64)
                for gb in range(2):
                    nc.sync.dma_start(
                        out=lt4[rows, :, gb, sp * 64 : sp * 64 + 64],
                        in_=av[sp, :, :, gb, :].bitcast(F32R),
                    )

            # ---- stage 2 + norm, per sample pair ----
            for s in range(NPAIR):
                xet = xe.tile([128, 2 * L], F32R, tag="xe")
                nc.sync.dma_start(
                    out=xet[:, 0:L], in_=xe_in[s, :, 0:L].bitcast(F32R)
                )
                nc.sync.dma_start(
                    out=xet[:, L : 2 * L], in_=xe_in[s, :, L : 2 * L].bitcast(F32R)
                )
                xt = xet[:, 0:L].bitcast(FP32)
                et = xet[:, L : 2 * L]

                st = stat.tile([128, 2, 6], FP32, tag="st")
                nc.vector.bn_stats(st[:, 0, :], xt[:, 0:512])
                nc.vector.bn_stats(st[:, 1, :], xt[:, 512:1024])
                mv = stat.tile([128, 2], FP32, tag="mv")
                nc.vector.bn_aggr(mv, st)
                rstd = stat.tile([128, 1], FP32, tag="rstd")
                nc.scalar.activation(
                    out=rstd, in_=mv[:, 1:2], func=AF.Sqrt, bias=eps_t, scale=1.0
                )
                nc.vector.reciprocal(rstd, rstd)

                msb = work.tile([128, L], FP32, tag="m")
                ot = work.tile([128, L], FP32, tag="o")
                pm = ps.tile([128, L], FP32, tag="ps", name=f"pm{s}")
                pb = ps.tile([128, L], FP32, tag="ps", name=f"pb{s}")
                for n in range(2):
                    cols = slice(n * 512, (n + 1) * 512)
                    pmh = pm[:, cols]
                    pbh = pb[:, cols]
                    nc.tensor.matmul(
                        pmh,
                        lhsT=lt_all[:, s, 0:128],
                        rhs=et[:, cols],
                        start=True,
                        stop=False,
                    )
                    nc.tensor.matmul(
                        pmh,
                        lhsT=l2[:, 0:128],
                        rhs=r2[:, s, cols],
                        start=False,
                        stop=True,
                    )
                    nc.tensor.matmul(
                        pbh,
                        lhsT=lt_all[:, s, 128:256],
                        rhs=et[:, cols],
                        start=True,
                        stop=False,
                    )
                    nc.tensor.matmul(
                        pbh,
                        lhsT=l2[:, 128:256],
                        rhs=r2[:, s, cols],
                        start=False,
                        stop=True,
                    )
                    # evacuate + fold rstd:  M'' = (1+gamma) * rstd
                    nc.scalar.activation(
                        out=msb[:, cols], in_=pmh, func=AF.Copy, scale=rstd
                    )
                    # ot = (x - mean) * M''   (fused),  then += beta
                    nc.vector.scalar_tensor_tensor(
                        out=ot[:, cols],
                        in0=xt[:, cols],
                        scalar=mv[:, 0:1],
                        in1=msb[:, cols],
                        op0=OP.subtract,
                        op1=OP.mult,
                    )
                    nc.vector.tensor_add(
                        out=ot[:, cols], in0=ot[:, cols], in1=pbh
                    )

                od = out_d.rearrange("b c (h l) -> b c h l", h=2)
                for n in range(2):
                    nc.sync.dma_start(
                        out=od[2 * s : 2 * s + 2, :, n, :],
                        in_=ot[:, n * 512 : (n + 1) * 512],
                    )

    return nc


def _prep_core_inputs(core, x, u_i, e_qid, t, W, V, bias):
    bg, cg = divmod(core, CG)
    bs = slice(bg * BPC, (bg + 1) * BPC)
    rg = slice(cg * CPC, (cg + 1) * CPC)
    rb = slice(C + cg * CPC, C + (cg + 1) * CPC)

    # xe: (NPAIR, 128, 2048) = [x pair rows | e pair rows]
    xp = x[bs, rg, :].reshape(NPAIR, 128, L)
    ep = e_qid[bs].reshape(NPAIR, 128, L)
    xe = np.concatenate([xp, ep], axis=2)

    w2 = np.concatenate([W[rg], W[rb]], axis=0)          # (128, 4096) c2=[g|b]
    wr = w2.reshape(128, U, 2, 32)                       # [c2, u, qh, ql]
    wt2 = np.ascontiguousarray(wr.transpose(2, 1, 3, 0)).reshape(128, 4096)

    ui_s = np.ascontiguousarray(u_i[bs].T)               # (64, 8)
    ui2 = np.concatenate([ui_s, ui_s], axis=0)           # (128, 8)
    wt2 = np.concatenate([ui2, wt2], axis=1)             # (128, 8+4096)

    vg, vb = V[rg, 0], V[rb, 0]
    bgm, bbt = bias[rg], bias[rb]
    l2 = np.zeros((3, 256), np.float32)
    l2[0, 0:64] = vg
    l2[1, 64:128] = vg
    l2[2, 0:64] = 1.0 + bgm
    l2[2, 64:128] = 1.0 + bgm
    l2[0, 128:192] = vb
    l2[1, 192:256] = vb
    l2[2, 128:192] = bbt
    l2[2, 192:256] = bbt

    r2 = np.empty((3, NPAIR, L), np.float32)
    for s in range(NPAIR):
        r2[0, s] = t[bg * BPC + 2 * s, 0]
        r2[1, s] = t[bg * BPC + 2 * s + 1, 0]
    r2[2] = 1.0
    sm = np.concatenate([r2.reshape(3, NPAIR * L), l2], axis=1)

    return {
        "xe_s": np.ascontiguousarray(xe, dtype=np.float32),
        "wt2": wt2.astype(np.float32),
        "sm2": np.ascontiguousarray(sm, dtype=np.float32),
    }


def kernel(x, u_i, e_qid, t, W, V, bias):
    x = np.asarray(x, np.float32)
    u_i = np.asarray(u_i, np.float32)
    e_qid = np.asarray(e_qid, np.float32)
    t = np.asarray(t, np.float32)
    W = np.asarray(W, np.float32)
    V = np.asarray(V, np.float32)
    bias = np.asarray(bias, np.float32)

    if "nc" not in _CACHE:
        _CACHE["nc"] = _build_nc()
    nc = _CACHE["nc"]

    in_maps = [
        _prep_core_inputs(i, x, u_i, e_qid, t, W, V, bias) for i in range(N_CORES)
    ]
    results = run_bass_kernel_spmd(nc, in_maps, list(range(N_CORES))).results

    out = np.empty((B, C, L), np.float32)
    for i in range(N_CORES):
        bg, cg = divmod(i, CG)
        out[bg * BPC : (bg + 1) * BPC, cg * CPC : (cg + 1) * CPC, :] = results[i][
            "out_s"
        ]
    return out



# revision 21
# speedup vs baseline: 1.5348x; 1.5348x over previous
"""ConditionAwareAdaIN Trainium2 kernel (v3).

Reference computation (B=16, C=256, L=1024, U=64, Q=64):
    nx    = InstanceNorm1d(x)                       # per-(b,c) stats over L
    A     = einsum('bu,cuq->bcq', u_i, W.reshape(2C,U,Q))
    style = einsum('bcq,bql->bcl', A, e_qid)
    gamma, beta = split(style + V@t + bias, 2, axis=1)
    out   = (1 + gamma) * nx + beta
Sharding: 2-way batch x 4-way channels -> 8 cores (8 samples x 64 ch each,
owning both the gamma and beta rows of W/V/bias for its channels).

Everything streamed is bf16 (tolerance 2e-2; ends up ~1e-3).  The DMA issue
path (HWDGE, ~625ns/DMA, serialized) and the DMA bus are both scarce, so
input count/bytes are minimized: 10 input DMAs, ~3.1 MB.

Device kernel per core:
  stats:   all pairs up-front on DVE/ACT while inputs stream: bn_stats/
           bn_aggr, rstd, mr = mean*rstd, and xr = x*rstd (ACT, per-
           partition scale) so the later pointwise needs no msb evac.
  stage 1: flipped A matmuls: 66 matmuls [M=128 c2, N=8 b] with K=65
           (64 u rows + a ones row); two extra "q" slots carry V and the
           (1+bias | bias) row through the same path, so the stage-2 lhsT
           block a_lt[q'=66, b, c2] comes entirely out of the transposes.
  redistribute: one ACT evac [128, 528] -> 16 PE transposes (identity from
           affine_select) -> per-pair ACT evacs into a_lt. No DRAM bounce.
  stage 2: per pair, 8 matmuls K=66: psum Pg = 1+gamma_full (pair-stacked
           rows: 2 samples x 64 ch), Pb = beta_full.
  pointwise: ot = (xr - mr) * Pg  (stt; h0 on DVE, h1 on Pool, reading
           gamma psum directly), out = ot + Pb (h0 Pool, h1 DVE), bf16 out
           DMA per half, upcast on host.
  PE pstate: interleaved warm-up matmuls keep the tensor engine gapless so
           it ramps to and holds the 2.4 GHz pstate for stage 2.
"""

import json

import numpy as np
import ml_dtypes

for _p in ("/opt/trn_rl_repo", "/root/.axon_site/_ro/trn_rl_repo"):
    import sys as _sys

    if _p not in _sys.path:
        _sys.path.append(_p)

import concourse.bass as bass
import concourse.mybir as mybir
from concourse.tile import TileContext
from concourse.bass_utils import run_bass_kernel_spmd

BF16NP = ml_dtypes.bfloat16


def _split_sync_waits(raw: bytes, keep: int = 1) -> bytes:
    """Walrus in this env accepts at most one sync wait per TPB instruction.

    Tile packs several waits into sync_info.on_wait; re-emit the excess as
    standalone single-wait EventSemaphore instructions (what wait_ge emits)
    immediately before the instruction, in the same engine stream.
    """
    bir = json.loads(raw)
    n = 0
    for fn in bir["functions"]:
        for blk in fn["blocks"]:
            out = []
            for ins in blk["instructions"]:
                si = ins.get("sync_info")
                ws = si.get("on_wait") if si else None
                if ws and len(ws) > keep:
                    for w in ws[: len(ws) - keep]:
                        n += 1
                        out.append(
                            {
                                "debug": ins.get("debug", 0),
                                "engine": ins["engine"],
                                "ins": [],
                                "outs": [],
                                "name": f"evw-{n}",
                                "opcode": "EventSemaphore",
                                "sync_info": {"on_update": [], "on_wait": [w]},
                            }
                        )
                    si["on_wait"] = ws[len(ws) - keep :]
                out.append(ins)
            blk["instructions"] = out
    return json.dumps(bir).encode()


class _Bass(bass.Bass):
    def to_json_bytes(self) -> bytes:
        return _split_sync_waits(super().to_json_bytes())


B, C, L = 16, 256, 1024
U, Q = 64, 64
QX = Q + 2             # q' slots: 64 real q + V slot + bias slot
EPS = 1e-5
N_CORES = 8
BG, CG = 2, 4          # batch groups x channel groups
BPC = B // BG          # samples per core = 8
CPC = C // CG          # channels per core = 64
NPAIR = BPC // 2       # sample pairs per core = 4

FP32 = mybir.dt.float32
BF16 = mybir.dt.bfloat16

_CACHE = {}


def _build_nc(detect_races: bool = True):
    nc = _Bass(detect_race_conditions=detect_races)

    AF = mybir.ActivationFunctionType
    OP = mybir.AluOpType

    # host-packed inputs (all bf16)
    #   wt: [65 (u | ones), 8 (u_i^T | ones) + 66 q' * 128 c2]
    wt_in = nc.dram_tensor("wt3", [U + 1, BPC + QX * 128], BF16, kind="ExternalInput")
    #   e': [66 q', 8 b, 1024 l]  rows 0:64 e_qid, row 64 t, row 65 ones
    e_in = nc.dram_tensor("ep3", [QX, BPC, L], BF16, kind="ExternalInput")
    #   x:  [128 (sp,c), 4 pair, 1024 l]
    x_in = nc.dram_tensor("xp3", [128, NPAIR, L], BF16, kind="ExternalInput")
    out_d = nc.dram_tensor("out_s", [NPAIR, 128, L], BF16, kind="ExternalOutput")

    with TileContext(nc) as tc:
        with (
            tc.tile_pool(name="persist", bufs=1) as persist,
            tc.tile_pool(name="stat", bufs=8) as stat,
            tc.tile_pool(name="work", bufs=4) as work,
            tc.tile_pool(name="wups", bufs=1, space="PSUM") as wups,
        ):
            # ---- on-device constants ----
            wusrc = persist.tile([128, 512], BF16, tag="wusrc")
            nc.gpsimd.memset(wusrc.bitcast(FP32), 0.0)
            eps_t = persist.tile([128, 1], FP32, tag="eps")
            nc.vector.memset(eps_t, EPS)
            ones_t = persist.tile([128, 128], BF16, tag="ones_t")
            idt = persist.tile([128, 128], BF16, tag="idt")

            # streamed inputs.  DMA order = service order: wt chunks feed the
            # stage-1 critical chain; x01 early starts the stats pipelines;
            # e chunks gate stage-2 pairs in order.
            wt = persist.tile([U + 1, BPC + QX * 128], BF16, tag="wt")
            e_all = persist.tile([QX, BPC, L], BF16, tag="e_all")
            x_all = persist.tile([128, NPAIR, L], BF16, tag="x_all")

            qsplit = [0, 17, 34, 50, QX]

            def wt_chunk(ck):
                c0 = 0 if ck == 0 else BPC + qsplit[ck] * 128
                c1 = BPC + qsplit[ck + 1] * 128
                nc.sync.dma_start(out=wt[:, c0:c1], in_=wt_in[:, c0:c1])

            wt_chunk(0)
            wt_chunk(1)
            wt_chunk(2)
            nc.sync.dma_start(out=x_all[:, 0:1, :], in_=x_in[:, 0:1, :])
            wt_chunk(3)
            nc.sync.dma_start(out=x_all[:, 1:2, :], in_=x_in[:, 1:2, :])
            nc.sync.dma_start(out=x_all[:, 2:3, :], in_=x_in[:, 2:3, :])
            nc.sync.dma_start(out=e_all[:, 0:2, :], in_=e_in[:, 0:2, :])
            nc.sync.dma_start(out=x_all[:, 3:4, :], in_=x_in[:, 3:4, :])
            nc.sync.dma_start(out=e_all[:, 2:4, :], in_=e_in[:, 2:4, :])
            nc.sync.dma_start(out=e_all[:, 4:6, :], in_=e_in[:, 4:6, :])
            nc.sync.dma_start(out=e_all[:, 6:8, :], in_=e_in[:, 6:8, :])
            ui = wt[:, 0:BPC]

            wu_ps = wups.tile([128, 512], FP32, tag="wu")

            def wu(i):
                nc.tensor.matmul(
                    wu_ps, lhsT=wusrc[:, 0:128], rhs=wusrc[:, 0:512],
                    start=True, stop=True,
                )

            nwu = 0

            # identity for the PE transposes: ones tile -> keep diagonal
            nc.gpsimd.memset(ones_t, 1.0)
            nc.gpsimd.affine_select(
                out=idt, in_=ones_t, pattern=[[1, 128]],
                compare_op=OP.is_equal, fill=0.0, channel_multiplier=-1, base=0,
            )

            # ---- DVE-side stats, all pairs up-front (DVE idle while
            # inputs stream; each chain starts as soon as its x chunk lands).
            # ACT-side sqrt + the per-sample rstd scale vectors are deferred
            # past the stage-1 evac to avoid head-of-line blocking ACT.
            mvs = [None] * NPAIR
            rstds = [None] * NPAIR

            # s_all[:, b]: per-sample column-scale for the transposes:
            # rows 0:64 (gamma cols) = rstd[b], rows 64:128 (beta cols) = 1
            s_all = persist.tile([128, BPC], FP32, tag="s_all")
            nc.vector.memset(s_all[64:128, :], 1.0)

            def stats(p):
                xt = x_all[:, p, :]
                st = stat.tile([128, 2, 6], FP32, tag="st")
                nc.vector.bn_stats(st[:, 0, :], xt[:, 0:512])
                nc.vector.bn_stats(st[:, 1, :], xt[:, 512:1024])
                mv = stat.tile([128, 2], FP32, tag="mv")
                nc.vector.bn_aggr(mv, st)
                mvs[p] = mv
                rstd = stat.tile([128, 1], FP32, tag="rstd")
                nc.scalar.activation(
                    out=rstd, in_=mv[:, 1:2], func=AF.Sqrt, bias=eps_t, scale=1.0
                )
                rstds[p] = rstd
                nc.vector.reciprocal(rstd, rstd)
                for sp in range(2):
                    b = 2 * p + sp
                    nc.vector.tensor_copy(
                        out=s_all[0:64, b : b + 1],
                        in_=rstd[sp * 64 : sp * 64 + 64, 0:1],
                    )

            for p in range(NPAIR):
                stats(p)

            # ---- stage 1 (flipped): pA[c2, q'*8+b] = sum_u' wt[u',q',c2] ui[u',b]
            aT = persist.tile([128, QX * BPC], BF16, tag="aT")
            with tc.tile_pool(name="ps1", bufs=1, space="PSUM") as ps1:
                pA = ps1.tile([128, QX * BPC], FP32, tag="pA")
                for i in range(6):
                    nwu += 1
                    wu(nwu)
                for q in range(QX):
                    nc.tensor.matmul(
                        pA[:, q * BPC : (q + 1) * BPC],
                        lhsT=wt[:, BPC + q * 128 : BPC + (q + 1) * 128],
                        rhs=ui,
                        start=True,
                        stop=True,
                    )
                    if q % 6 == 5:
                        nwu += 1
                        wu(nwu)
                # evac A^T (c2-major) to SBUF bf16
                nc.scalar.activation(out=aT, in_=pA, func=AF.Copy)
                for i in range(4):
                    nwu += 1
                    wu(nwu)

            # ---- per-pair lhsT prep + stage 2 + pointwise, pipelined ----
            # prep(p): scale aT columns of each sample by s_all[:, b] (ACT;
            # folds rstd into the gamma half of the lhsT), PE-transpose
            # [128 c2, 66 q'] -> [66, 128] via identity, evac to a_lt (ACT).
            # Then per pair: Pg = rstd*(1+gamma_full) pair-stacked, Pb =
            # beta_full; ot = (x - mean) * Pg (stt h0 DVE / h1 Pool);
            # out = ot + Pb (h0 Pool / h1 DVE); bf16 DMA per half.
            a_lt = persist.tile([QX, BPC, 128], BF16, tag="a_lt")
            aTs = persist.tile([128, BPC, QX], BF16, tag="aTs")
            aTv = aT.rearrange("p (q b) -> p q b", b=BPC)
            with (
                tc.tile_pool(name="ps1b", bufs=1, space="PSUM") as ps1b,
                tc.tile_pool(name="ps2", bufs=6, space="PSUM") as ps2,
            ):
                pT = ps1b.tile([QX, BPC * 128], BF16, tag="pT")
                pTv = pT.rearrange("p (b c) -> p b c", c=128)

                def prep(p):
                    for sp in range(2):
                        b = 2 * p + sp
                        nc.scalar.activation(
                            out=aTs[:, b, :], in_=aTv[:, :, b], func=AF.Copy,
                            scale=s_all[:, b : b + 1],
                        )
                        nc.tensor.transpose(
                            out=pT[:, b * 128 : (b + 1) * 128],
                            in_=aTs[:, b, :],
                            identity=idt,
                        )
                    nc.scalar.activation(
                        out=a_lt[:, 2 * p : 2 * p + 2, :],
                        in_=pTv[:, 2 * p : 2 * p + 2, :],
                        func=AF.Copy,
                    )

                pg = {}
                pb = {}
                ots = {}

                def mm_group(p, half, pp):
                    b0, b1 = 2 * p, 2 * p + 1
                    cc = slice(half * 64, half * 64 + 64)
                    for h in range(2):
                        cols = slice(h * 512, (h + 1) * 512)
                        nc.tensor.matmul(
                            pp[h][0:64, :], lhsT=a_lt[:, b0, cc],
                            rhs=e_all[:, b0, cols], start=True, stop=True,
                        )
                        nc.tensor.matmul(
                            pp[h][64:128, :], lhsT=a_lt[:, b1, cc],
                            rhs=e_all[:, b1, cols], start=True, stop=True,
                        )

                def gamma(p):
                    pg[p] = [
                        ps2.tile([128, 512], FP32, tag="s2", name=f"pg{p}h{h}")
                        for h in range(2)
                    ]
                    mm_group(p, 0, pg[p])

                def beta(p):
                    pb[p] = [
                        ps2.tile([128, 512], FP32, tag="s2", name=f"pb{p}h{h}")
                        for h in range(2)
                    ]
                    mm_group(p, 1, pb[p])

                def stt(p):
                    # ot = (x - mean) * Pg   (DVE only: Pool cannot read PSUM)
                    ot = work.tile([128, L], BF16, tag="ot")
                    ots[p] = ot
                    for h in range(2):
                        cols = slice(h * 512, (h + 1) * 512)
                        nc.vector.scalar_tensor_tensor(
                            out=ot[:, cols], in0=x_all[:, p, cols],
                            scalar=mvs[p][:, 0:1],
                            in1=pg[p][h], op0=OP.subtract, op1=OP.mult,
                        )

                def finish(p):
                    # pairs 0/2: evac beta psum on ACT, add on Pool (SBUF);
                    # pairs 1/3: add directly from psum on DVE
                    outb = work.tile([128, L], BF16, tag="outb")
                    if p in (0, 2):
                        bsb = work.tile([128, L], BF16, tag="bsb", bufs=2)
                        for h in range(2):
                            cols = slice(h * 512, (h + 1) * 512)
                            nc.scalar.activation(out=bsb[:, cols], in_=pb[p][h], func=AF.Copy)
                            nc.gpsimd.tensor_add(
                                out=outb[:, cols], in0=ots[p][:, cols], in1=bsb[:, cols]
                            )
                            nc.sync.dma_start(out=out_d[p, :, cols], in_=outb[:, cols])
                    else:
                        for h in range(2):
                            cols = slice(h * 512, (h + 1) * 512)
                            nc.vector.tensor_add(
                                out=outb[:, cols], in0=ots[p][:, cols], in1=pb[p][h]
                            )
                            nc.sync.dma_start(out=out_d[p, :, cols], in_=outb[:, cols])

                prep(0)
                nwu += 1
                wu(nwu)
                prep(1)
                gamma(0)
                stt(0)
                beta(0)
                gamma(1)
                prep(2)
                finish(0)
                stt(1)
                beta(1)
                prep(3)
                gamma(2)
                finish(1)
                stt(2)
                gamma(3)
                stt(3)
                beta(2)
                finish(2)
                beta(3)
                finish(3)

    return nc


def _prep_core_inputs(core, x, u_i, e_qid, t, W, V, bias):
    bg, cg = divmod(core, CG)
    bs = slice(bg * BPC, (bg + 1) * BPC)
    rg = slice(cg * CPC, (cg + 1) * CPC)
    rb = slice(C + cg * CPC, C + (cg + 1) * CPC)

    # wt: [65, 8 + 66*128]: row 64 = ones (for u_i part) / const slots
    w2 = np.concatenate([W[rg], W[rb]], axis=0)          # (128 c2, 4096)
    wr = w2.reshape(128, U, Q)                           # [c2, u, q]
    wt = np.zeros((U + 1, BPC + QX * 128), np.float32)
    wt[0:U, 0:BPC] = u_i[bs].T
    wt[U, 0:BPC] = 1.0
    wt[0:U, BPC : BPC + Q * 128] = wr.transpose(1, 2, 0).reshape(U, Q * 128)
    wt[U, BPC + Q * 128 : BPC + (Q + 1) * 128] = np.concatenate([V[rg, 0], V[rb, 0]])
    wt[U, BPC + (Q + 1) * 128 :] = np.concatenate([1.0 + bias[rg], bias[rb]])

    # e': [66, 8, 1024]
    ep = np.empty((QX, BPC, L), np.float32)
    ep[0:Q] = e_qid[bs].transpose(1, 0, 2)
    ep[Q] = t[bs][:, 0, :]
    ep[Q + 1] = 1.0

    # x: [128 (sp,c), 4 pair, 1024]
    xp = (
        x[bs, rg, :]
        .reshape(NPAIR, 2, CPC, L)
        .transpose(1, 2, 0, 3)
        .reshape(128, NPAIR, L)
    )

    return {
        "wt3": wt.astype(BF16NP),
        "ep3": ep.astype(BF16NP),
        "xp3": xp.astype(BF16NP),
    }


def kernel(x, u_i, e_qid, t, W, V, bias):
    x = np.asarray(x, np.float32)
    u_i = np.asarray(u_i, np.float32)
    e_qid = np.asarray(e_qid, np.float32)
    t = np.asarray(t, np.float32)
    W = np.asarray(W, np.float32)
    V = np.asarray(V, np.float32)
    bias = np.asarray(bias, np.float32)

    if "nc" not in _CACHE:
        _CACHE["nc"] = _build_nc()
    nc = _CACHE["nc"]

    in_maps = [
        _prep_core_inputs(i, x, u_i, e_qid, t, W, V, bias) for i in range(N_CORES)
    ]
    results = run_bass_kernel_spmd(nc, in_maps, list(range(N_CORES))).results

    out = np.empty((B, C, L), np.float32)
    for i in range(N_CORES):
        bg, cg = divmod(i, CG)
        blk = np.asarray(results[i]["out_s"]).astype(np.float32)
        out[bg * BPC : (bg + 1) * BPC, cg * CPC : (cg + 1) * CPC, :] = blk.reshape(
            BPC, CPC, L
        )
    return out


# revision 23
# speedup vs baseline: 1.5876x; 1.0344x over previous
"""ConditionAwareAdaIN Trainium2 kernel (v3).

Reference computation (B=16, C=256, L=1024, U=64, Q=64):
    nx    = InstanceNorm1d(x)                       # per-(b,c) stats over L
    A     = einsum('bu,cuq->bcq', u_i, W.reshape(2C,U,Q))
    style = einsum('bcq,bql->bcl', A, e_qid)
    gamma, beta = split(style + V@t + bias, 2, axis=1)
    out   = (1 + gamma) * nx + beta
Sharding: 2-way batch x 4-way channels -> 8 cores (8 samples x 64 ch each,
owning both the gamma and beta rows of W/V/bias for its channels).

Everything streamed is bf16 (tolerance 2e-2; ends up ~1e-3).  The DMA issue
path (HWDGE, ~625ns/DMA, serialized) and the DMA bus are both scarce, so
input count/bytes are minimized: 10 input DMAs, ~3.1 MB.

Device kernel per core:
  stats:   all pairs up-front on DVE/ACT while inputs stream: bn_stats/
           bn_aggr, rstd, mr = mean*rstd, and xr = x*rstd (ACT, per-
           partition scale) so the later pointwise needs no msb evac.
  stage 1: flipped A matmuls: 66 matmuls [M=128 c2, N=8 b] with K=65
           (64 u rows + a ones row); two extra "q" slots carry V and the
           (1+bias | bias) row through the same path, so the stage-2 lhsT
           block a_lt[q'=66, b, c2] comes entirely out of the transposes.
  redistribute: one ACT evac [128, 528] -> 16 PE transposes (identity from
           affine_select) -> per-pair ACT evacs into a_lt. No DRAM bounce.
  stage 2: per pair, 8 matmuls K=66: psum Pg = 1+gamma_full (pair-stacked
           rows: 2 samples x 64 ch), Pb = beta_full.
  pointwise: ot = (xr - mr) * Pg  (stt; h0 on DVE, h1 on Pool, reading
           gamma psum directly), out = ot + Pb (h0 Pool, h1 DVE), bf16 out
           DMA per half, upcast on host.
  PE pstate: interleaved warm-up matmuls keep the tensor engine gapless so
           it ramps to and holds the 2.4 GHz pstate for stage 2.
"""

import json

import numpy as np
import ml_dtypes

for _p in ("/opt/trn_rl_repo", "/root/.axon_site/_ro/trn_rl_repo"):
    import sys as _sys

    if _p not in _sys.path:
        _sys.path.append(_p)

import concourse.bass as bass
import concourse.mybir as mybir
from concourse.tile import TileContext
from concourse.bass_utils import run_bass_kernel_spmd

BF16NP = ml_dtypes.bfloat16


def _split_sync_waits(raw: bytes, keep: int = 1) -> bytes:
    """Walrus in this env accepts at most one sync wait per TPB instruction.

    Tile packs several waits into sync_info.on_wait; re-emit the excess as
    standalone single-wait EventSemaphore instructions (what wait_ge emits)
    immediately before the instruction, in the same engine stream.
    """
    bir = json.loads(raw)
    n = 0
    for fn in bir["functions"]:
        for blk in fn["blocks"]:
            out = []
            for ins in blk["instructions"]:
                si = ins.get("sync_info")
                ws = si.get("on_wait") if si else None
                if ws and len(ws) > keep:
                    for w in ws[: len(ws) - keep]:
                        n += 1
                        out.append(
                            {
                                "debug": ins.get("debug", 0),
                                "engine": ins["engine"],
                                "ins": [],
                                "outs": [],
                                "name": f"evw-{n}",
                                "opcode": "EventSemaphore",
                                "sync_info": {"on_update": [], "on_wait": [w]},
                            }
                        )
                    si["on_wait"] = ws[len(ws) - keep :]
                out.append(ins)
            blk["instructions"] = out
    return json.dumps(bir).encode()


class _Bass(bass.Bass):
    def to_json_bytes(self) -> bytes:
        return _split_sync_waits(super().to_json_bytes())


B, C, L = 16, 256, 1024
U, Q = 64, 64
QX = Q + 2             # q' slots: 64 real q + V slot + bias slot
EPS = 1e-5
N_CORES = 8
BG, CG = 2, 4          # batch groups x channel groups
BPC = B // BG          # samples per core = 8
CPC = C // CG          # channels per core = 64
NPAIR = BPC // 2       # sample pairs per core = 4

FP32 = mybir.dt.float32
BF16 = mybir.dt.bfloat16

_CACHE = {}


def _build_nc(detect_races: bool = True):
    nc = _Bass(detect_race_conditions=detect_races)

    AF = mybir.ActivationFunctionType
    OP = mybir.AluOpType

    # host-packed inputs (all bf16)
    #   wt: [65 (u | ones), 8 (u_i^T | ones) + 66 q' * 128 c2]
    wt_in = nc.dram_tensor("wt3", [U + 1, BPC + QX * 128], BF16, kind="ExternalInput")
    #   e': [66 q', 8 b, 1024 l]  rows 0:64 e_qid, row 64 t, row 65 ones
    e_in = nc.dram_tensor("ep3", [QX, BPC, L], BF16, kind="ExternalInput")
    #   x:  [128 (sp,c), 4 pair, 1024 l]
    x_in = nc.dram_tensor("xp3", [128, NPAIR, L], BF16, kind="ExternalInput")
    out_d = nc.dram_tensor("out_s", [NPAIR, 128, L], BF16, kind="ExternalOutput")

    with TileContext(nc) as tc:
        with (
            tc.tile_pool(name="persist", bufs=1) as persist,
            tc.tile_pool(name="stat", bufs=8) as stat,
            tc.tile_pool(name="work", bufs=4) as work,
            tc.tile_pool(name="wups", bufs=1, space="PSUM") as wups,
        ):
            # ---- on-device constants ----
            wusrc = persist.tile([128, 512], BF16, tag="wusrc")
            nc.gpsimd.memset(wusrc.bitcast(FP32), 0.0)
            eps_t = persist.tile([128, 1], FP32, tag="eps")
            nc.vector.memset(eps_t, EPS)
            ones_t = persist.tile([128, 128], BF16, tag="ones_t")
            idt = persist.tile([128, 128], BF16, tag="idt")

            # streamed inputs.  DMA order = service order: wt chunks feed the
            # stage-1 critical chain; x01 early starts the stats pipelines;
            # e chunks gate stage-2 pairs in order.
            wt = persist.tile([U + 1, BPC + QX * 128], BF16, tag="wt")
            e_all = persist.tile([QX, BPC, L], BF16, tag="e_all")
            x_all = persist.tile([128, NPAIR, L], BF16, tag="x_all")

            qsplit = [0, 17, 34, 50, QX]

            def wt_chunk(ck):
                c0 = 0 if ck == 0 else BPC + qsplit[ck] * 128
                c1 = BPC + qsplit[ck + 1] * 128
                nc.sync.dma_start(out=wt[:, c0:c1], in_=wt_in[:, c0:c1])

            wt_chunk(0)
            wt_chunk(1)
            wt_chunk(2)
            nc.sync.dma_start(out=x_all[:, 0:1, :], in_=x_in[:, 0:1, :])
            wt_chunk(3)
            nc.sync.dma_start(out=x_all[:, 1:2, :], in_=x_in[:, 1:2, :])
            nc.sync.dma_start(out=x_all[:, 2:3, :], in_=x_in[:, 2:3, :])
            nc.sync.dma_start(out=e_all[:, 0:2, :], in_=e_in[:, 0:2, :])
            nc.sync.dma_start(out=x_all[:, 3:4, :], in_=x_in[:, 3:4, :])
            nc.sync.dma_start(out=e_all[:, 2:4, :], in_=e_in[:, 2:4, :])
            nc.sync.dma_start(out=e_all[:, 4:6, :], in_=e_in[:, 4:6, :])
            nc.sync.dma_start(out=e_all[:, 6:8, :], in_=e_in[:, 6:8, :])
            ui = wt[:, 0:BPC]

            wu_ps = wups.tile([128, 512], FP32, tag="wu")

            def wu(i):
                nc.tensor.matmul(
                    wu_ps, lhsT=wusrc[:, 0:128], rhs=wusrc[:, 0:512],
                    start=True, stop=True,
                )

            nwu = 0

            # identity for the PE transposes: ones tile -> keep diagonal
            nc.gpsimd.memset(ones_t, 1.0)
            nc.gpsimd.affine_select(
                out=idt, in_=ones_t, pattern=[[1, 128]],
                compare_op=OP.is_equal, fill=0.0, channel_multiplier=-1, base=0,
            )

            # ---- DVE-side stats, all pairs up-front (DVE idle while
            # inputs stream; each chain starts as soon as its x chunk lands).
            # ACT-side sqrt + the per-sample rstd scale vectors are deferred
            # past the stage-1 evac to avoid head-of-line blocking ACT.
            mvs = [None] * NPAIR
            rstds = [None] * NPAIR

            # s_all[:, b]: per-sample column-scale for the transposes:
            # rows 0:64 (gamma cols) = rstd[b], rows 64:128 (beta cols) = 1
            s_all = persist.tile([128, BPC], FP32, tag="s_all")
            nc.vector.memset(s_all[64:128, :], 1.0)

            def stats(p):
                xt = x_all[:, p, :]
                st = stat.tile([128, 2, 6], FP32, tag="st")
                nc.vector.bn_stats(st[:, 0, :], xt[:, 0:512])
                nc.vector.bn_stats(st[:, 1, :], xt[:, 512:1024])
                mv = stat.tile([128, 2], FP32, tag="mv")
                nc.vector.bn_aggr(mv, st)
                mvs[p] = mv
                rstd = stat.tile([128, 1], FP32, tag="rstd")
                nc.scalar.activation(
                    out=rstd, in_=mv[:, 1:2], func=AF.Sqrt, bias=eps_t, scale=1.0
                )
                rstds[p] = rstd
                nc.vector.reciprocal(rstd, rstd)
                for sp in range(2):
                    b = 2 * p + sp
                    nc.vector.tensor_copy(
                        out=s_all[0:64, b : b + 1],
                        in_=rstd[sp * 64 : sp * 64 + 64, 0:1],
                    )

            for p in range(NPAIR):
                stats(p)

            # ---- stage 1 (flipped): pA[c2, q'*8+b] = sum_u' wt[u',q',c2] ui[u',b]
            aT = persist.tile([128, QX * BPC], BF16, tag="aT")
            with tc.tile_pool(name="ps1", bufs=1, space="PSUM") as ps1:
                pA = ps1.tile([128, QX * BPC], FP32, tag="pA")
                for i in range(6):
                    nwu += 1
                    wu(nwu)
                for q in range(QX):
                    nc.tensor.matmul(
                        pA[:, q * BPC : (q + 1) * BPC],
                        lhsT=wt[:, BPC + q * 128 : BPC + (q + 1) * 128],
                        rhs=ui,
                        start=True,
                        stop=True,
                    )
                    if q % 6 == 5:
                        nwu += 1
                        wu(nwu)
                # evac A^T (c2-major) to SBUF bf16
                nc.scalar.activation(out=aT, in_=pA, func=AF.Copy)
                for i in range(4):
                    nwu += 1
                    wu(nwu)

            # ---- per-pair lhsT prep + stage 2 + pointwise, pipelined ----
            # prep(p): scale aT columns of each sample by s_all[:, b] (ACT;
            # folds rstd into the gamma half of the lhsT), PE-transpose
            # [128 c2, 66 q'] -> [66, 128] via identity, evac to a_lt (ACT).
            # Then per pair: Pg = rstd*(1+gamma_full) pair-stacked, Pb =
            # beta_full; ot = (x - mean) * Pg (stt h0 DVE / h1 Pool);
            # out = ot + Pb (h0 Pool / h1 DVE); bf16 DMA per half.
            a_lt = persist.tile([QX, BPC, 128], BF16, tag="a_lt")
            aTs = persist.tile([128, BPC, QX], BF16, tag="aTs")
            aTv = aT.rearrange("p (q b) -> p q b", b=BPC)
            with (
                tc.tile_pool(name="ps1b", bufs=1, space="PSUM") as ps1b,
                tc.tile_pool(name="ps2", bufs=6, space="PSUM") as ps2,
            ):
                pT = ps1b.tile([QX, BPC * 128], BF16, tag="pT")
                pTv = pT.rearrange("p (b c) -> p b c", c=128)

                def prep(p):
                    for sp in range(2):
                        b = 2 * p + sp
                        nc.scalar.activation(
                            out=aTs[:, b, :], in_=aTv[:, :, b], func=AF.Copy,
                            scale=s_all[:, b : b + 1],
                        )
                        nc.tensor.transpose(
                            out=pT[:, b * 128 : (b + 1) * 128],
                            in_=aTs[:, b, :],
                            identity=idt,
                        )
                    nc.scalar.activation(
                        out=a_lt[:, 2 * p : 2 * p + 2, :],
                        in_=pTv[:, 2 * p : 2 * p + 2, :],
                        func=AF.Copy,
                    )

                pg = {}
                pb = {}
                ots = {}

                def mm_group(p, half, pp):
                    b0, b1 = 2 * p, 2 * p + 1
                    cc = slice(half * 64, half * 64 + 64)
                    for h in range(2):
                        cols = slice(h * 512, (h + 1) * 512)
                        nc.tensor.matmul(
                            pp[h][0:64, :], lhsT=a_lt[:, b0, cc],
                            rhs=e_all[:, b0, cols], start=True, stop=True,
                        )
                        nc.tensor.matmul(
                            pp[h][64:128, :], lhsT=a_lt[:, b1, cc],
                            rhs=e_all[:, b1, cols], start=True, stop=True,
                        )

                def gamma(p):
                    pg[p] = [
                        ps2.tile([128, 512], FP32, tag="s2", name=f"pg{p}h{h}")
                        for h in range(2)
                    ]
                    mm_group(p, 0, pg[p])

                def beta(p):
                    pb[p] = [
                        ps2.tile([128, 512], FP32, tag="s2", name=f"pb{p}h{h}")
                        for h in range(2)
                    ]
                    mm_group(p, 1, pb[p])

                def stt(p):
                    # ot = (x - mean) * Pg   (DVE only: Pool cannot read PSUM)
                    ot = work.tile([128, L], BF16, tag="ot")
                    ots[p] = ot
                    for h in range(2):
                        cols = slice(h * 512, (h + 1) * 512)
                        nc.vector.scalar_tensor_tensor(
                            out=ot[:, cols], in0=x_all[:, p, cols],
                            scalar=mvs[p][:, 0:1],
                            in1=pg[p][h], op0=OP.subtract, op1=OP.mult,
                        )

                def finish(p):
                    # pairs 0/2: evac beta psum on ACT, add on Pool (SBUF);
                    # pairs 1/3: add directly from psum on DVE
                    outb = work.tile([128, L], BF16, tag="outb")
                    if p in (0, 1):
                        bsb = work.tile([128, L], BF16, tag="bsb", bufs=2)
                        for h in range(2):
                            cols = slice(h * 512, (h + 1) * 512)
                            nc.scalar.activation(out=bsb[:, cols], in_=pb[p][h], func=AF.Copy)
                            nc.gpsimd.tensor_add(
                                out=outb[:, cols], in0=ots[p][:, cols], in1=bsb[:, cols]
                            )
                            nc.sync.dma_start(out=out_d[p, :, cols], in_=outb[:, cols])
                    else:
                        for h in range(2):
                            cols = slice(h * 512, (h + 1) * 512)
                            nc.vector.tensor_add(
                                out=outb[:, cols], in0=ots[p][:, cols], in1=pb[p][h]
                            )
                            nc.sync.dma_start(out=out_d[p, :, cols], in_=outb[:, cols])

                prep(0)
                nwu += 1
                wu(nwu)
                prep(1)
                gamma(0)
                stt(0)
                beta(0)
                gamma(1)
                prep(2)
                finish(0)
                stt(1)
                beta(1)
                prep(3)
                gamma(2)
                finish(1)
                stt(2)
                gamma(3)
                stt(3)
                beta(3)
                finish(3)
                beta(2)
                finish(2)

    return nc


def _prep_core_inputs(core, x, u_i, e_qid, t, W, V, bias):
    bg, cg = divmod(core, CG)
    bs = slice(bg * BPC, (bg + 1) * BPC)
    rg = slice(cg * CPC, (cg + 1) * CPC)
    rb = slice(C + cg * CPC, C + (cg + 1) * CPC)

    # wt: [65, 8 + 66*128]: row 64 = ones (for u_i part) / const slots
    w2 = np.concatenate([W[rg], W[rb]], axis=0)          # (128 c2, 4096)
    wr = w2.reshape(128, U, Q)                           # [c2, u, q]
    wt = np.zeros((U + 1, BPC + QX * 128), np.float32)
    wt[0:U, 0:BPC] = u_i[bs].T
    wt[U, 0:BPC] = 1.0
    wt[0:U, BPC : BPC + Q * 128] = wr.transpose(1, 2, 0).reshape(U, Q * 128)
    wt[U, BPC + Q * 128 : BPC + (Q + 1) * 128] = np.concatenate([V[rg, 0], V[rb, 0]])
    wt[U, BPC + (Q + 1) * 128 :] = np.concatenate([1.0 + bias[rg], bias[rb]])

    # e': [66, 8, 1024]
    ep = np.empty((QX, BPC, L), np.float32)
    ep[0:Q] = e_qid[bs].transpose(1, 0, 2)
    ep[Q] = t[bs][:, 0, :]
    ep[Q + 1] = 1.0

    # x: [128 (sp,c), 4 pair, 1024]
    xp = (
        x[bs, rg, :]
        .reshape(NPAIR, 2, CPC, L)
        .transpose(1, 2, 0, 3)
        .reshape(128, NPAIR, L)
    )

    return {
        "wt3": wt.astype(BF16NP),
        "ep3": ep.astype(BF16NP),
        "xp3": xp.astype(BF16NP),
    }


def kernel(x, u_i, e_qid, t, W, V, bias):
    x = np.asarray(x, np.float32)
    u_i = np.asarray(u_i, np.float32)
    e_qid = np.asarray(e_qid, np.float32)
    t = np.asarray(t, np.float32)
    W = np.asarray(W, np.float32)
    V = np.asarray(V, np.float32)
    bias = np.asarray(bias, np.float32)

    if "nc" not in _CACHE:
        _CACHE["nc"] = _build_nc()
    nc = _CACHE["nc"]

    in_maps = [
        _prep_core_inputs(i, x, u_i, e_qid, t, W, V, bias) for i in range(N_CORES)
    ]
    results = run_bass_kernel_spmd(nc, in_maps, list(range(N_CORES))).results

    out = np.empty((B, C, L), np.float32)
    for i in range(N_CORES):
        bg, cg = divmod(i, CG)
        blk = np.asarray(results[i]["out_s"]).astype(np.float32)
        out[bg * BPC : (bg + 1) * BPC, cg * CPC : (cg + 1) * CPC, :] = blk.reshape(
            BPC, CPC, L
        )
    return out
